# revision 2
# baseline (speedup 1.0000x reference)
"""Dependency-GCN via host pre-gather + fp16 matmul + dma_scatter_add
for 8 Trainium2 NeuronCores.

Strategy (single SPMD program, no collectives):
  - Each core owns a contiguous range of 3750 destination nodes; edges
    are routed to their dst-owner core (fwd: dep, rev: gov).
  - Host pre-combines edges sharing (direction, relation, dst): their
    source rows are summed on the host, so each (direction, relation)
    group has at most ONE cell per dst.  Cells are packed into 128-wide
    blocks grouped by (relation, dst-half) pieces.
  - The gather is done ON THE HOST: x_blocks [nblk, 128, 256] fp16
    holds, for block b, the transposed source features laid out as
    (k-partition, k-half*128 + edge) so a plain contiguous DMA load
    yields ready-to-use matmul lhsT tiles.  No SWDGE gather, no
    transposer, no device-side index tables for the input side.
  - Per block: two fp16 matmuls (K=256 as 2 k-tiles of 128) accumulate
    into PSUM; FWL hides the LDWEIGHTS since lhsT is 128 cols of fp16.
    One PSUM->SBUF fp32->fp16 copy per 2 blocks (alternating
    Activation/DVE engines).
  - dma_scatter_add loses updates for duplicate indices WITHIN one
    instruction (measured on HW) but separate instructions on a queue
    serialize; the (dir, rel) grouping makes every piece duplicate-
    free by construction.  To overlap the per-piece WAW chains, dst
    rows are split into two contiguous halves of the out tensor (each
    with its own trash row for pads) scattered on two different SWDGE
    queues: ranges are disjoint, so the two chains pipeline.
  - out [2*1876, 256] fp16 (A rows 0..1874, A trash, B rows, B trash)
    is initialized by a DRAM->DRAM DMA from a host-precomputed bias
    image (b_self + indeg_r @ b_fwd/b_rev); the self transform rides
    the pipeline as relation 20 (unique dsts).
"""

import sys

if "/opt/trn_rl_repo" not in sys.path:
    sys.path.insert(0, "/opt/trn_rl_repo")

import os as _os
import numpy as np

import concourse.bacc as bacc
import concourse.mybir as mybir
from concourse.tile import TileContext
from concourse.bass_utils import run_bass_kernel_spmd

F32 = mybir.dt.float32
F16 = mybir.dt.float16
I16 = mybir.dt.int16

N_NODES = 30000
N_REL = 10
D = 256
N_CORES = 8
NODES_PER_CORE = N_NODES // N_CORES          # 3750
HALF = NODES_PER_CORE // 2                    # 1875 rows per half
HROWS = HALF + 1                              # + trash row
SELF_REL = 20
GBC = int(_os.environ.get("GCN_GBC", "8"))   # blocks per load chunk


# ---------------------------------------------------------------- host prep

def _pack_idx16(idx: np.ndarray) -> np.ndarray:
    """[n] -> [128, n//16] int16: idx i at (partition i%16, col i//16), x8."""
    n = idx.shape[0]
    t = idx.astype(np.int16).reshape(n // 16, 16).T
    return np.tile(t, (8, 1))


def prepare(x, W_self, b_self, W_fwd, b_fwd, W_rev, b_rev,
            dep_idx, rel_idx, gov_idx):
    dep_idx = np.asarray(dep_idx).astype(np.int64)
    rel_idx = np.asarray(rel_idx).astype(np.int64)
    gov_idx = np.asarray(gov_idx).astype(np.int64)
    x = np.asarray(x, np.float32)
    x16 = x.astype(np.float16)

    # weight stack [128, 2, 21, 256] fp16: dim1 = k-tile half
    W_all = np.zeros((21, D, D), np.float32)
    W_all[0:10] = np.asarray(W_fwd, np.float32)
    W_all[10:20] = np.asarray(W_rev, np.float32)
    W_all[20] = np.asarray(W_self, np.float32)
    wsb = np.zeros((128, 2, 21, D), np.float16)
    for h in range(2):
        wsb[:, h, :, :] = W_all[:, h * 128:(h + 1) * 128, :].transpose(
            1, 0, 2).astype(np.float16)

    # ---- per-core edges keyed by (relW, local dst); dedupe cells
    core_key = [[] for _ in range(N_CORES)]
    core_src = [[] for _ in range(N_CORES)]
    for d in range(2):
        if d == 0:
            src_a, dst_a, relw_a = gov_idx, dep_idx, rel_idx
        else:
            src_a, dst_a, relw_a = dep_idx, gov_idx, rel_idx + 10
        core_of = dst_a // NODES_PER_CORE
        for c in range(N_CORES):
            m = core_of == c
            core_key[c].append(relw_a[m] * NODES_PER_CORE
                               + (dst_a[m] - c * NODES_PER_CORE))
            core_src[c].append(src_a[m])

    per_core = []
    max_cells = np.zeros((20, 2), np.int64)   # per (relW, half) over cores
    for c in range(N_CORES):
        key = np.concatenate(core_key[c])
        src = np.concatenate(core_src[c])
        order = np.argsort(key, kind="stable")
        key, src = key[order], src[order]
        ukey, start, cnt = np.unique(key, return_index=True,
                                     return_counts=True)
        n_u = ukey.shape[0]
        single = cnt == 1
        multi = np.nonzero(~single)[0]
        comb_rows = np.zeros((len(multi), D), np.float32)
        for j, ui in enumerate(multi):
            s = start[ui]
            comb_rows[j] = x[src[s:s + cnt[ui]]].sum(0)
        gsrc = np.empty(n_u, np.int64)
        gsrc[single] = src[start[single]]
        gsrc[~single] = N_NODES + np.arange(len(multi))
        relw = ukey // NODES_PER_CORE
        dstl = ukey % NODES_PER_CORE
        half = (dstl >= HALF).astype(np.int64)
        cells = {}
        for rw in range(20):
            for h in range(2):
                m = (relw == rw) & (half == h)
                cells[(rw, h)] = (dstl[m] - h * HALF, gsrc[m])
                max_cells[rw, h] = max(max_cells[rw, h], int(m.sum()))
        per_core.append((cells, comb_rows.astype(np.float16)))

    # schedule: pieces (relW, half, nblocks, n_real); self placement via env
    nblk_self = (HALF + 127) // 128               # 15
    selfp = [(SELF_REL, 0, nblk_self), (SELF_REL, 1, nblk_self)]
    relp = []
    for rw in range(20):
        for h in range(2):
            nb = (int(max_cells[rw, h]) + 127) // 128
            if nb > 0:
                relp.append((rw, h, nb))
    pos = _os.environ.get("GCN_SELFPOS", "first")
    if pos == "first":
        pieces = selfp + relp
    elif pos == "last":
        pieces = relp + selfp
    else:  # mid
        k = int(pos)
        pieces = relp[:k] + selfp + relp[k:]
    nblk_total = sum(p[2] for p in pieces)
    # pad block count to a chunk multiple so every load is full-size
    nblk_pad = (nblk_total + GBC - 1) // GBC * GBC

    in_maps = []
    for c in range(N_CORES):
        cells, comb16 = per_core[c]
        # source table: global fp16 rows, then this core's combined rows,
        # then one zero row for pads
        table = np.concatenate(
            [x16, comb16, np.zeros((1, D), np.float16)], axis=0)
        zrow = table.shape[0] - 1

        src_all = np.full(nblk_pad * 128, zrow, np.int64)
        sidx = np.full(nblk_total * 128, HALF, np.int16)  # trash (local)
        bi = 0
        for (rw, h, nb) in pieces:
            s0 = bi * 128
            if rw == SELF_REL:
                n_real = HALF
                sidx[s0:s0 + n_real] = np.arange(HALF, dtype=np.int16)
                src_all[s0:s0 + n_real] = (c * NODES_PER_CORE + h * HALF
                                           + np.arange(HALF))
            else:
                dstl, gs = cells[(rw, h)]
                n_real = dstl.shape[0]
                sidx[s0:s0 + n_real] = dstl.astype(np.int16)
                src_all[s0:s0 + n_real] = gs
            bi += nb
        assert bi == nblk_total

        # host gather + transpose into matmul-ready block layout:
        # x_blocks[b, p, j*128+e] = feature (p + 128j) of edge e of block b
        A = table[src_all].reshape(nblk_pad, 128, 2, 128)   # [b, e, j, p]
        x_blocks = np.ascontiguousarray(A.transpose(0, 3, 2, 1)).reshape(
            nblk_pad, 128, 256)

        # bias image [2*HROWS, 256] fp16 in half-local layout
        lo = c * NODES_PER_CORE
        hi = lo + NODES_PER_CORE
        cnt_f = np.zeros((NODES_PER_CORE, N_REL), np.float32)
        mf = (dep_idx >= lo) & (dep_idx < hi)
        np.add.at(cnt_f, (dep_idx[mf] - lo, rel_idx[mf]), 1.0)
        cnt_r = np.zeros((NODES_PER_CORE, N_REL), np.float32)
        mr = (gov_idx >= lo) & (gov_idx < hi)
        np.add.at(cnt_r, (gov_idx[mr] - lo, rel_idx[mr]), 1.0)
        bias = (np.asarray(b_self, np.float32)[None, :]
                + cnt_f @ np.asarray(b_fwd, np.float32)
                + cnt_r @ np.asarray(b_rev, np.float32))
        binit = np.zeros((2 * HROWS, D), np.float32)
        binit[0:HALF] = bias[0:HALF]
        binit[HROWS:HROWS + HALF] = bias[HALF:]
        in_maps.append({
            "x_blocks": x_blocks,
            "wsb": wsb,
            "sidx": _pack_idx16(sidx),
            "binit": binit.astype(np.float16),
        })

    return pieces, nblk_total, nblk_pad, in_maps


# ---------------------------------------------------------------- device

def build_bass(pieces, nblk_total, nblk_pad):
    nc = bacc.Bacc(num_swdge_queues=2)
    x_blocks = nc.declare_dram_parameter("x_blocks", [nblk_pad, 128, 256],
                                         F16, isOutput=False)
    wsb = nc.declare_dram_parameter("wsb", [128, 2, 21, D], F16,
                                    isOutput=False)
    sidx = nc.declare_dram_parameter("sidx", [128, nblk_total * 8], I16,
                                     isOutput=False)
    binit = nc.declare_dram_parameter("binit", [2 * HROWS, D], F16,
                                      isOutput=False)
    out = nc.declare_dram_parameter("out", [2 * HROWS, D], F16,
                                    isOutput=True)

    n_ch = nblk_pad // GBC
    # per-piece n_real (actual scatter rows; pads beyond are trash)
    n_reals = []
    for (rw, h, nb) in pieces:
        n_reals.append(HALF if rw == SELF_REL else None)

    with TileContext(nc) as tc:
        with (
            tc.tile_pool(name="cst", bufs=1) as cst,
            tc.tile_pool(name="xp", bufs=int(_os.environ.get("GCN_XPB", "6"))) as xp,
            tc.tile_pool(name="sp", bufs=int(_os.environ.get("GCN_SPB", "4"))) as sp,
            tc.tile_pool(name="pm", bufs=4, space="PSUM") as pm,
        ):
            wsb_t = cst.tile([128, 2, 21, D], F16, tag="wsb")
            nc.sync.dma_start(out=wsb_t[:], in_=wsb[:])

            chunks = [None] * n_ch

            def issue_load(j):
                if j >= n_ch or chunks[j] is not None:
                    return
                ch = xp.tile([128, GBC, 256], F16, tag="x")
                nc.sync.dma_start(
                    out=ch[:],
                    in_=x_blocks[j * GBC:(j + 1) * GBC, :, :].rearrange(
                        "b p c -> p b c"))
                chunks[j] = ch

            issue_load(0)
            issue_load(1)
            issue_load(2)

            # out init: DRAM->DRAM copies of the bias image (per half so
            # the halves' WAW chains stay independent)
            nc.sync.dma_start(out=out[0:HROWS, :], in_=binit[0:HROWS, :])
            nc.sync.dma_start(out=out[HROWS:2 * HROWS, :],
                              in_=binit[HROWS:2 * HROWS, :])
            sidx_t = cst.tile([128, nblk_total * 8], I16, tag="sidx")
            nc.sync.dma_start(out=sidx_t[:], in_=sidx[:])

            reps = int(_os.environ.get("GCN_REPS", "1"))
            copy_i = 0
            for _rep in range(reps):
              if _rep > 0:
                chunks[:] = [None] * n_ch
                issue_load(0)
                issue_load(1)
                issue_load(2)
                nc.sync.dma_start(out=out[0:HROWS, :],
                                  in_=binit[0:HROWS, :])
                nc.sync.dma_start(out=out[HROWS:2 * HROWS, :],
                                  in_=binit[HROWS:2 * HROWS, :])
              bi = 0
              for (rw, h, nb) in pieces:
                  n_real = HALF if rw == SELF_REL else None
                  msg = sp.tile([128, nb, D], F16, tag="msg")
                  k = 0
                  while k < nb:
                      # pair up to 2 blocks in one PSUM bank -> one copy
                      kn = min(2, nb - k)
                      m_ps = pm.tile([128, 2, D], F32, tag="m")
                      for j in range(kn):
                          b = bi + k + j
                          if b % GBC == 0:
                              issue_load(b // GBC + 3)
                          ch = chunks[b // GBC]
                          s = b % GBC
                          nc.tensor.matmul(
                              out=m_ps[:, j, :],
                              lhsT=ch[:, s, 0:128],
                              rhs=wsb_t[:, 0, rw, :],
                              start=True, stop=False)
                          nc.tensor.matmul(
                              out=m_ps[:, j, :],
                              lhsT=ch[:, s, 128:256],
                              rhs=wsb_t[:, 1, rw, :],
                              start=False, stop=True)
                      if copy_i % 2 == 0:
                          nc.scalar.copy(out=msg[:, k:k + kn, :],
                                         in_=m_ps[:, 0:kn, :])
                      else:
                          nc.vector.tensor_copy(msg[:, k:k + kn, :],
                                                m_ps[:, 0:kn, :])
                      copy_i += 1
                      k += kn
                  nc.gpsimd.dma_scatter_add(
                      out_ap=out[h * HROWS:(h + 1) * HROWS, :],
                      in_ap=msg[:],
                      idxs_ap=sidx_t[:, bi * 8:(bi + nb) * 8],
                      num_idxs=nb * 128,
                      num_idxs_reg=nb * 128,
                      elem_size=D,
                      queue_num=h,
                  )
                  bi += nb
    nc.finalize()
    return nc


# ---------------------------------------------------------------- entry

def kernel(x, W_self, b_self, W_fwd, b_fwd, W_rev, b_rev,
           dep_idx, rel_idx, gov_idx, _trace=False, _trace_kwargs=None):
    pieces, nblk_total, nblk_pad, in_maps = prepare(
        x, W_self, b_self, W_fwd, b_fwd, W_rev, b_rev,
        dep_idx, rel_idx, gov_idx)
    nc = build_bass(pieces, nblk_total, nblk_pad)
    res = run_bass_kernel_spmd(nc, in_maps, list(range(N_CORES)),
                               trace=_trace, **(_trace_kwargs or {}))
    outs = []
    for c in range(N_CORES):
        o = res.results[c]["out"]
        outs.append(o[0:HALF])
        outs.append(o[HROWS:HROWS + HALF])
    kernel._last_results = res
    return np.concatenate(outs, axis=0).astype(np.float32)


# revision 8
# speedup vs baseline: 5.1112x; 5.1112x over previous
"""Dependency-GCN via host pre-gather + per-window PSUM accumulation
for 8 Trainium2 NeuronCores.  No scatter, no SWDGE, no collectives.

Strategy (single SPMD program):
  - Each core owns a contiguous range of 3750 destination nodes; edges
    are routed to their dst-owner core (fwd: dep, rev: gov).
  - Host pre-combines edges sharing (direction, relation, dst): their
    source rows are summed on the host, so each (direction, relation)
    group has at most ONE cell per dst.
  - Destinations are grouped into 30 windows of 128.  For window w and
    relation-weight r (20 edge rels + self as rel 20), a 128-column
    lhsT block holds the cell source features at column = dst % 128
    (zero columns where the (r, dst) cell is absent).  The 21 rel
    blocks of a window accumulate into ONE PSUM tile via matmul
    accumulation -- the "scatter" happens positionally in PSUM.
  - Bias rides as a k=21 matmul per window: lhsT = per-dst edge counts
    for each rel (+ const-1 row), rhs = [b_fwd; b_rev; b_self].  This
    reproduces out += cnt_r * b_r exactly (multi-edge cells carry
    their edge count).
  - The gather is done ON THE HOST: x_blocks [128, nblk*256] fp16
    holds, for block b, the transposed source features laid out as
    (k-partition, b*256 + k_half*128 + column) so plain contiguous
    DMA loads (multi-KB descriptor runs) yield ready-to-use matmul
    lhsT tiles.
  - Per window: 43 fp16 matmuls (FWL hides weight loads) -> one
    PSUM->SBUF fp32->fp16 copy (alternating Activation/DVE) -> one
    plain contiguous DMA write of the finished 128 output rows.
"""

import sys

if "/opt/trn_rl_repo" not in sys.path:
    sys.path.insert(0, "/opt/trn_rl_repo")

import os as _os
import numpy as np

import concourse.bacc as bacc
import concourse.mybir as mybir
from concourse.tile import TileContext
from concourse.bass_utils import run_bass_kernel_spmd

F32 = mybir.dt.float32
F16 = mybir.dt.float16

N_NODES = 30000
N_REL = 10
D = 256
N_CORES = 8
NODES_PER_CORE = N_NODES // N_CORES          # 3750
NW = (NODES_PER_CORE + 127) // 128            # 30 windows of 128 dsts
NRW = 21                                      # 20 edge rels + self
GBC = int(_os.environ.get("GCN_GBC", "8"))   # blocks per load chunk


# ---------------------------------------------------------------- host prep

def prepare(x, W_self, b_self, W_fwd, b_fwd, W_rev, b_rev,
            dep_idx, rel_idx, gov_idx):
    dep_idx = np.asarray(dep_idx).astype(np.int64)
    rel_idx = np.asarray(rel_idx).astype(np.int64)
    gov_idx = np.asarray(gov_idx).astype(np.int64)
    x = np.asarray(x, np.float32)
    x16 = x.astype(np.float16)

    # weight stack [128, 2, 21, 256] fp16: dim1 = k-tile half
    W_all = np.zeros((NRW, D, D), np.float32)
    W_all[0:10] = np.asarray(W_fwd, np.float32)
    W_all[10:20] = np.asarray(W_rev, np.float32)
    W_all[20] = np.asarray(W_self, np.float32)
    wsb = np.zeros((128, 2, NRW, D), np.float16)
    for h in range(2):
        wsb[:, h, :, :] = W_all[:, h * 128:(h + 1) * 128, :].transpose(
            1, 0, 2).astype(np.float16)

    # bias table [21, 256] fp16
    ball = np.concatenate(
        [np.asarray(b_fwd, np.float32),
         np.asarray(b_rev, np.float32),
         np.asarray(b_self, np.float32)[None, :]], axis=0).astype(np.float16)

    nblk = NW * NRW
    nblk_pad = (nblk + GBC - 1) // GBC * GBC

    # ---- per-core edges keyed by (relW, local dst); dedupe cells
    core_key = [[] for _ in range(N_CORES)]
    core_src = [[] for _ in range(N_CORES)]
    for d in range(2):
        if d == 0:
            src_a, dst_a, relw_a = gov_idx, dep_idx, rel_idx
        else:
            src_a, dst_a, relw_a = dep_idx, gov_idx, rel_idx + 10
        core_of = dst_a // NODES_PER_CORE
        for c in range(N_CORES):
            m = core_of == c
            core_key[c].append(relw_a[m] * NODES_PER_CORE
                               + (dst_a[m] - c * NODES_PER_CORE))
            core_src[c].append(src_a[m])

    in_maps = []
    for c in range(N_CORES):
        key = np.concatenate(core_key[c])
        src = np.concatenate(core_src[c])
        order = np.argsort(key, kind="stable")
        key, src = key[order], src[order]
        ukey, start, cnt = np.unique(key, return_index=True,
                                     return_counts=True)
        single = cnt == 1
        multi = np.nonzero(~single)[0]
        comb_rows = np.zeros((len(multi), D), np.float32)
        for j, ui in enumerate(multi):
            s = start[ui]
            comb_rows[j] = x[src[s:s + cnt[ui]]].sum(0)
        gsrc = np.empty(ukey.shape[0], np.int64)
        gsrc[single] = src[start[single]]
        gsrc[~single] = N_NODES + np.arange(len(multi))
        relw = ukey // NODES_PER_CORE
        dstl = ukey % NODES_PER_CORE

        table = np.concatenate(
            [x16, comb_rows.astype(np.float16),
             np.zeros((1, D), np.float16)], axis=0)
        zrow = table.shape[0] - 1

        # block b = w*21 + r; column = dstl % 128
        src_all = np.full(nblk_pad * 128, zrow, np.int64)
        w_arr = dstl // 128
        pos = dstl % 128
        src_all[(w_arr * NRW + relw) * 128 + pos] = gsrc
        # self blocks: r = 20, every real dst
        dl = np.arange(NODES_PER_CORE)
        src_all[((dl // 128) * NRW + 20) * 128 + dl % 128] = \
            c * NODES_PER_CORE + dl

        # cnt table [21, NW*128] fp16: per-dst edge counts + const row
        cntb = np.zeros((NRW, NW * 128), np.float16)
        cntb[relw, w_arr * 128 + pos] = cnt.astype(np.float16)
        cntb[20, :NODES_PER_CORE] = 1.0

        # host gather + transpose into matmul-ready flat layout:
        # x_blocks[p, b*256 + j*128 + e] = feat (p + 128j) of col e of blk b
        A = table[src_all].reshape(nblk_pad, 128, 2, 128)   # [b, e, j, p]
        x_blocks = np.ascontiguousarray(
            A.transpose(3, 0, 2, 1)).reshape(128, nblk_pad * 256)

        in_maps.append({
            "x_blocks": x_blocks,
            "wsb": wsb,
            "ball": ball,
            "cntb": cntb,
        })

    return NW, nblk, nblk_pad, in_maps


# ---------------------------------------------------------------- device

def build_bass(nw, nblk, nblk_pad):
    nc = bacc.Bacc()
    x_blocks = nc.declare_dram_parameter("x_blocks", [128, nblk_pad * 256],
                                         F16, isOutput=False)
    wsb = nc.declare_dram_parameter("wsb", [128, 2, NRW, D], F16,
                                    isOutput=False)
    ball = nc.declare_dram_parameter("ball", [NRW, D], F16, isOutput=False)
    cntb = nc.declare_dram_parameter("cntb", [NRW, nw * 128], F16,
                                     isOutput=False)
    out = nc.declare_dram_parameter("out", [nw * 128, D], F16,
                                    isOutput=True)

    n_ch = nblk_pad // GBC

    with TileContext(nc) as tc:
        with (
            tc.tile_pool(name="cst", bufs=1) as cst,
            tc.tile_pool(name="xp", bufs=int(_os.environ.get("GCN_XPB", "6"))) as xp,
            tc.tile_pool(name="ot", bufs=4) as ot,
            tc.tile_pool(name="pm",
                         bufs=int(_os.environ.get("GCN_PMB", "6")),
                         space="PSUM") as pm,
        ):
            wsb_t = cst.tile([128, 2, NRW, D], F16, tag="wsb")
            nc.sync.dma_start(out=wsb_t[:], in_=wsb[:])
            ball_t = cst.tile([NRW, D], F16, tag="ball")
            nc.sync.dma_start(out=ball_t[:], in_=ball[:])
            cntb_t = cst.tile([NRW, nw * 128], F16, tag="cntb")
            nc.sync.dma_start(out=cntb_t[:], in_=cntb[:])

            chunks = [None] * n_ch

            def issue_load(j):
                if j >= n_ch or chunks[j] is not None:
                    return
                ch = xp.tile([128, GBC * 256], F16, tag="x")
                nc.sync.dma_start(
                    out=ch[:],
                    in_=x_blocks[:, j * GBC * 256:(j + 1) * GBC * 256])
                chunks[j] = ch

            reps = int(_os.environ.get("GCN_REPS", "1"))
            for _rep in range(reps):
                chunks[:] = [None] * n_ch
                issue_load(0)
                issue_load(1)
                issue_load(2)
                for w in range(nw):
                    ps = pm.tile([128, D], F32, tag="ps")
                    nc.tensor.matmul(
                        out=ps[:],
                        lhsT=cntb_t[:, w * 128:(w + 1) * 128],
                        rhs=ball_t[:],
                        start=True, stop=False)
                    for r in range(NRW):
                        b = w * NRW + r
                        if b % GBC == 0:
                            issue_load(b // GBC + 3)
                        ch = chunks[b // GBC]
                        s = (b % GBC) * 256
                        nc.tensor.matmul(
                            out=ps[:],
                            lhsT=ch[:, s:s + 128],
                            rhs=wsb_t[:, 0, r, :],
                            start=False, stop=False)
                        nc.tensor.matmul(
                            out=ps[:],
                            lhsT=ch[:, s + 128:s + 256],
                            rhs=wsb_t[:, 1, r, :],
                            start=False, stop=(r == NRW - 1))
                    o_t = ot.tile([128, D], F16, tag="o")
                    if w % 2 == 0:
                        nc.scalar.copy(out=o_t[:], in_=ps[:])
                    else:
                        nc.vector.tensor_copy(o_t[:], ps[:])
                    nc.sync.dma_start(out=out[w * 128:(w + 1) * 128, :],
                                      in_=o_t[:])
    nc.finalize()
    return nc


# ---------------------------------------------------------------- entry

def kernel(x, W_self, b_self, W_fwd, b_fwd, W_rev, b_rev,
           dep_idx, rel_idx, gov_idx, _trace=False, _trace_kwargs=None):
    nw, nblk, nblk_pad, in_maps = prepare(
        x, W_self, b_self, W_fwd, b_fwd, W_rev, b_rev,
        dep_idx, rel_idx, gov_idx)
    nc = build_bass(nw, nblk, nblk_pad)
    res = run_bass_kernel_spmd(nc, in_maps, list(range(N_CORES)),
                               trace=_trace, **(_trace_kwargs or {}))
    outs = [res.results[c]["out"][0:NODES_PER_CORE] for c in range(N_CORES)]
    kernel._last_results = res
    return np.concatenate(outs, axis=0).astype(np.float32)
